# revision 15
# baseline (speedup 1.0000x reference)
"""Trainium2 Bass kernel for the MCA (multi-axis pooled gating) module.

Computation (per sample b):
    hw_m = mean_{u,v} x   uv_m = mean_{h,w} x   uh_m = mean_{v,w} x   vw_m = mean_{u,h} x
    body = conv2(silu(conv1(uvhw)))   (1x1 convs on the packed (H+V, W+U) pooled map)
    gates: hw_g = f0(body_hw), uv_g = f1(body_uv), uh_g = f2(body_uh), vw_g = f3(body_vw)
    out = x * (hw_g + uv_g + uh_g + vw_g)      (each gate broadcast to the 6D shape)

Distribution: 8 cores = 4 samples x 2 h-halves. Each core owns
x[b, :, :, :, hh*32:(hh+1)*32, :] (13.1 MB) resident in SBUF, so x is read from
HBM exactly once. The only cross-core data are the h-reduced pools
(uv_m, vw_m partials): an 88 KB pair AllReduce that overlaps with the hw-branch
convolutions.

On-core layout: SBUF partition p = hs*64 + c, where the core's 32 h-rows split
as h2 = hs*16 + hl. Pools that fully reduce h fold the hs partition halves with
a small DMA+add before the collective.

Engines: TensorE does the (u,v)-pool accumulation and the gate-broadcast
assembly in PSUM via float32r identity matmuls; VectorE does the w-reduction
and the single full-size multiply x*G; ScalarE handles pool scaling, SiLU and
bias adds; the collective runs on TOPSP.
"""

import sys
if '/opt/trn_rl_repo' not in sys.path:
    sys.path.insert(0, '/opt/trn_rl_repo')

from contextlib import ExitStack

import numpy as np
import concourse.bass as bass
import concourse.bacc as bacc
import concourse.tile as tile
from concourse import mybir

F32 = mybir.dt.float32
F32R = mybir.dt.float32r
AF = mybir.ActivationFunctionType
ALU = mybir.AluOpType


def _ap(t_ap, dims, extra_off=0):
    """Manual free-dim view of an AP: dims = [(step_elems, count), ...]."""
    return bass.AP(
        tensor=t_ap.tensor,
        offset=t_ap.offset + extra_off,
        ap=[list(t_ap.ap[0])] + [[s, c] for (s, c) in dims],
    )


def build_program(C=64, U=5, V=5, H2=32, W=64, n_cores=8):
    """One SPMD program; per-core inputs select the (b, h-half) shard."""
    assert C == 64 and H2 % 2 == 0
    HL = H2 // 2              # h rows per hs partition group
    P = 2 * C                 # 128 partitions = (hs, c)
    CHW = HL * W              # free size of one (u,v) chunk per partition
    NMM = min(512, CHW)       # matmul moving-operand max (fp32)
    NUV = U * V
    NUVP = NUV + (NUV % 2)   # fp32r matmul needs even innermost counts

    nc = bacc.Bacc('TRN2', target_bir_lowering=False, debug=False,
                   enable_asserts=False, num_devices=n_cores)

    x_d = nc.dram_tensor("x", [C, U, V, H2, W], F32, kind="ExternalInput").ap()
    out_d = nc.dram_tensor("out", [C, U, V, H2, W], F32, kind="ExternalOutput").ap()
    ident_d = nc.dram_tensor("ident", [2 * C, 2 * C], F32, kind="ExternalInput").ap()
    wts = {}
    for nm in ("w1T", "w2T", "f0T", "f1T", "f2T", "f3T"):
        wts[nm] = nc.dram_tensor(nm, [C, C], F32, kind="ExternalInput").ap()
    for nm in ("b1", "b2", "fb0", "fb1", "fb2", "fb3"):
        wts[nm] = nc.dram_tensor(nm, [C, 1], F32, kind="ExternalInput").ap()

    # DRAM views indexed [v, hs] / [u, hs]; DMAs run per (v|u, hs) half
    x_v_view = x_d.rearrange("c u v (hs hl) w -> v hs c u hl w", hs=2)
    out_u_view = out_d.rearrange("c u v (hs hl) w -> u hs c v hl w", hs=2)

    def mm(out_ps, lhsT, rhs, start, stop):
        nc.tensor.matmul(out_ps, lhsT.bitcast(F32R), rhs.bitcast(F32R),
                         start=start, stop=stop)

    with tile.TileContext(nc) as tc, ExitStack() as ctx:
        consts = ctx.enter_context(tc.tile_pool(name="consts", bufs=1))
        xpool = ctx.enter_context(tc.tile_pool(name="x", bufs=V))
        sumu_pool = ctx.enter_context(tc.tile_pool(name="sumu", bufs=2))
        small = ctx.enter_context(tc.tile_pool(name="small", bufs=1))
        convp = ctx.enter_context(tc.tile_pool(name="convp", bufs=2))
        ppool = ctx.enter_context(tc.tile_pool(name="pp", bufs=2))
        opool = ctx.enter_context(tc.tile_pool(name="op", bufs=2))
        phase1_ctx = ExitStack()
        ps_acc = phase1_ctx.enter_context(tc.tile_pool(name="ps_acc", bufs=2, space="PSUM"))
        ps_hw = phase1_ctx.enter_context(tc.tile_pool(name="ps_hw", bufs=1, space="PSUM"))
        dram = ctx.enter_context(tc.tile_pool(name="dram", bufs=1, space="DRAM"))

        ident = consts.tile([P, P], F32)
        nc.sync.dma_start(out=ident[:].bitcast(F32R), in_=ident_d[:, :].bitcast(F32R))
        # weights replicated on both hs partition halves so conv matmuls can
        # pick an lhsT whose base partition matches the rhs half
        wt = {nm: consts.tile([P, C], F32, name=f"wt_{nm}", tag=f"wt_{nm}") for nm in
              ("w1T", "w2T", "f0T", "f1T", "f2T", "f3T")}
        bt = {nm: consts.tile([C, 1], F32, name=f"bt_{nm}", tag=f"bt_{nm}") for nm in
              ("b1", "b2", "fb0", "fb1", "fb2", "fb3")}
        for nm in wt:
            for hs in range(2):
                nc.sync.dma_start(out=wt[nm][hs * C:(hs + 1) * C].bitcast(F32R),
                                  in_=wts[nm][:, :].bitcast(F32R))
        for nm in bt:
            nc.sync.dma_start(out=bt[nm][:], in_=wts[nm][:, :])

        # ---------------- Phase 1: load x + pools -------------------------
        NPART = NUV + V * W   # [uv (v,u) | vw (v,w)] partial sums
        partials = small.tile([P, NPART], F32)
        s_w = small.tile([P, V, U, HL], F32)      # x summed over w
        xv_t = []
        hw_ps = ps_hw.tile([P, CHW], F32)         # x summed over (u, v)

        for v in range(V):
            xv = xpool.tile([P, U, HL, W], F32, tag="xv")
            xv_t.append(xv)
            for hs in range(2):
                nc.sync.dma_start(out=xv[hs * C:(hs + 1) * C].bitcast(F32R),
                                  in_=x_v_view[v, hs].bitcast(F32R))

            acc = ps_acc.tile([P, CHW], F32, tag="acc")   # sum over u, this v
            for u in range(U):
                rhs = xv[:, u].rearrange("p hl w -> p (hl w)")
                for j0 in range(0, CHW, NMM):
                    mm(acc[:, j0:j0 + NMM], ident[:], rhs[:, j0:j0 + NMM],
                       start=(u == 0), stop=(u == U - 1))
            # vw partial: reduce hl out of acc  -> [P, W]
            accv = acc[:].rearrange("p (hl w) -> p w hl", hl=HL)
            nc.vector.tensor_reduce(partials[:, NUV + v * W: NUV + (v + 1) * W],
                                    accv, axis=mybir.AxisListType.X, op=ALU.add)
            # s_w: reduce w out of xv -> [P, U, HL]
            nc.vector.tensor_reduce(s_w[:, v], xv[:],
                                    axis=mybir.AxisListType.X, op=ALU.add)
            # hw accumulation: acc (copied to SBUF) back through the PE
            sumu = sumu_pool.tile([P, CHW], F32, tag="sumu")
            nc.scalar.copy(out=sumu[:].bitcast(F32R), in_=acc[:])
            for j0 in range(0, CHW, NMM):
                mm(hw_ps[:, j0:j0 + NMM], ident[:], sumu[:, j0:j0 + NMM],
                   start=(v == 0), stop=(v == V - 1))

        # uv partial: [P, (v,u)] ; uh local sums: [P, (u,hl)]
        nc.vector.tensor_reduce(partials[:, 0:NUV], s_w[:],
                                axis=mybir.AxisListType.X, op=ALU.add)
        uh_raw = small.tile([P, U, HL], F32)
        swv = s_w[:].rearrange("p v u hl -> p u hl v")
        nc.vector.tensor_reduce(uh_raw[:], swv, axis=mybir.AxisListType.X,
                                op=ALU.add)
        uh_sc = small.tile([P, U, HL], F32)
        nc.scalar.activation(out=uh_sc[:].bitcast(F32R), in_=uh_raw[:],
                             func=AF.Copy, scale=1.0 / (V * W))

        # hw means (count U*V), PSUM -> SBUF
        hw_m = small.tile([P, CHW], F32)
        nc.scalar.activation(out=hw_m[:].bitcast(F32R), in_=hw_ps[:], func=AF.Copy,
                             scale=1.0 / NUV)
        phase1_ctx.close()   # frees the acc/hw PSUM banks for conv + gate pools
        ps_cv = ctx.enter_context(tc.tile_pool(name="ps_cv", bufs=3, space="PSUM"))
        ps_g = ctx.enter_context(tc.tile_pool(name="ps_g", bufs=2, space="PSUM"))

        # ------------- fold hs halves + pair AllReduce --------------------
        fold_tmp = small.tile([C, NPART], F32)
        nc.sync.dma_start(out=fold_tmp[:], in_=partials[C:2 * C, :])
        cc_in = small.tile([C, NPART], F32)
        nc.vector.tensor_add(cc_in[:], partials[0:C, :], fold_tmp[:])

        cc_in_d = dram.tile([C, NPART], F32)
        cc_out_d = dram.tile([C, NPART], F32)
        nc.sync.dma_start(out=cc_in_d[:], in_=cc_in[:])
        groups = [[2 * i, 2 * i + 1] for i in range(n_cores // 2)]
        nc.gpsimd.collective_compute(
            "AllReduce", ALU.add, replica_groups=groups,
            ins=[cc_in_d[:].opt()], outs=[cc_out_d[:].opt()])
        cc_out = small.tile([C, NPART], F32)
        nc.sync.dma_start(out=cc_out[:], in_=cc_out_d[:])

        # scale to means: uv count H*W (H = 2*H2 across the pair), vw count U*H
        H = 2 * H2
        # padded even for fp32r; the pad col reads (finite) vw data, discarded
        uv_sc = small.tile([C, NUVP], F32)       # (v,u) order
        nc.scalar.activation(out=uv_sc[:].bitcast(F32R), in_=cc_out[:, 0:NUVP],
                             func=AF.Copy, scale=1.0 / (H * W))
        vw_sc = small.tile([C, V * W], F32)      # (v,w) order
        nc.scalar.activation(out=vw_sc[:].bitcast(F32R), in_=cc_out[:, NUV:],
                             func=AF.Copy, scale=1.0 / (U * H))

        # ---------------- Phase 2: conv chain on pooled pixels ------------
        hwg = small.tile([P, CHW], F32)          # (hl, w) per (hs,c) partition
        uhg = small.tile([P, HL * U], F32)       # (hl, u) per (hs,c) partition
        vwg = small.tile([P, V * W], F32)        # (v, w), replicated over hs
        uvg = small.tile([P, NUV], F32)          # (v, u), replicated over hs

        def conv_chain(rhs_for_chunk, n, f_nm, fb_nm, store, hs=0):
            """body = conv2(silu(conv1(rhs))) ; gate = f(body); store(gate,j0,nn)."""
            w_sl = slice(hs * C, (hs + 1) * C)
            for j0 in range(0, n, NMM):
                nn = min(NMM, n - j0)
                rhs = rhs_for_chunk(j0, nn)
                ps1 = ps_cv.tile([C, nn], F32, tag="cv")
                mm(ps1[:], wt["w1T"][w_sl], rhs, start=True, stop=True)
                sig = convp.tile([C, nn], F32, tag="sig")
                nc.scalar.activation(out=sig[:], in_=ps1[:], func=AF.Sigmoid,
                                     bias=bt["b1"][:])
                lin = convp.tile([C, nn], F32, tag="lin")
                nc.scalar.activation(out=lin[:], in_=ps1[:], func=AF.Identity,
                                     bias=bt["b1"][:])
                a1 = convp.tile([C, nn], F32, tag="a1")
                nc.vector.tensor_mul(a1[:].bitcast(F32R), sig[:], lin[:])
                ps2 = ps_cv.tile([C, nn], F32, tag="cv")
                mm(ps2[:], wt["w2T"][0:C], a1[:], start=True, stop=True)
                body = convp.tile([C, nn], F32, tag="body")
                nc.scalar.activation(out=body[:].bitcast(F32R), in_=ps2[:],
                                     func=AF.Identity, bias=bt["b2"][:])
                ps3 = ps_cv.tile([C, nn], F32, tag="cv")
                mm(ps3[:], wt[f_nm][0:C], body[:], start=True, stop=True)
                gate = convp.tile([C, nn], F32, tag="gate")
                nc.scalar.activation(out=gate[:], in_=ps3[:], func=AF.Identity,
                                     bias=bt[fb_nm][:])
                store(gate, j0, nn)

        # hw region: per hs half, pixels (hl, w); no collective dependency
        for hs in range(2):
            src = hw_m[hs * C:(hs + 1) * C, :]
            conv_chain(lambda j0, nn, src=src: src[:, j0:j0 + nn], CHW,
                       "f0T", "fb0",
                       lambda gate, j0, nn, hs=hs: nc.sync.dma_start(
                           out=hwg[hs * C:(hs + 1) * C, j0:j0 + nn],
                           in_=gate[:]), hs=hs)

        # uh region: per hs half, pixels in (u, hl) order (1x1 conv is
        # pointwise, so pixel order is free; innermost hl is even for fp32r)
        for hs in range(2):
            base = uh_sc[hs * C:(hs + 1) * C]
            conv_chain(lambda j0, nn, base=base: base, HL * U, "f2T", "fb2",
                       lambda gate, j0, nn, hs=hs: nc.sync.dma_start(
                           out=uhg[hs * C:(hs + 1) * C, :], in_=gate[:]), hs=hs)

        # vw region: pixels (v, w); uv region: pixels (v, u); replicate over hs
        def store_vw(gate, j0, nn):
            nc.sync.dma_start(out=vwg[0:C, :], in_=gate[:])
            nc.sync.dma_start(out=vwg[C:2 * C, :], in_=gate[:])
        conv_chain(lambda j0, nn: vw_sc[:], V * W, "f3T", "fb3", store_vw)

        def store_uv(gate, j0, nn):
            nc.sync.dma_start(out=uvg[0:C, :], in_=gate[:, 0:NUV])
            nc.sync.dma_start(out=uvg[C:2 * C, :], in_=gate[:, 0:NUV])
        conv_chain(lambda j0, nn: uv_sc[:], NUVP, "f1T", "fb1", store_uv)

        # Q[p, v, u, w] = vwg[p, v, w] + uvg[p, v, u]
        qbuf = small.tile([P, V, U, W], F32)
        vw_b = _ap(vwg[:], [(W, V), (0, U), (1, W)])
        uv_b = _ap(uvg[:], [(U, V), (1, U), (0, W)])
        nc.vector.tensor_add(qbuf[:].bitcast(F32R), vw_b, uv_b)

        # ---------------- Phase 3: gates * x, store -----------------------
        for u in range(U):
            pbuf = ppool.tile([P, CHW], F32, tag="p")
            # P = hwg + uhg[:, u, hl] broadcast over w
            uh_b = _ap(uhg[:], [(1, HL), (0, W)], extra_off=u * HL)
            nc.vector.tensor_add(pbuf[:].bitcast(F32R), hwg[:], uh_b)
            out_t = opool.tile([P, V, HL, W], F32, tag="out")
            for v in range(V):
                g = ps_g.tile([P, CHW], F32, tag="g")
                for j0 in range(0, CHW, NMM):
                    mm(g[:, j0:j0 + NMM], ident[:], pbuf[:, j0:j0 + NMM],
                       start=True, stop=False)
                    qs = _ap(qbuf[:], [(0, NMM // W), (1, W)],
                             extra_off=(v * U + u) * W)
                    mm(g[:, j0:j0 + NMM], ident[:], qs, start=False, stop=True)
                xin = xv_t[v][:, u].rearrange("p hl w -> p (hl w)")
                nc.vector.tensor_mul(out_t[:, v].rearrange("p hl w -> p (hl w)"),
                                     xin, g[:])
            for hs in range(2):
                nc.sync.dma_start(out=out_u_view[u, hs],
                                  in_=out_t[hs * C:(hs + 1) * C])

    nc.compile()
    return nc


# ---------------------------------------------------------------------------
# Host entry point (full problem size, 8 cores)

B, C, U, V, H, W = 4, 64, 5, 5, 64, 64
H2 = H // 2

_prog_cache = {}


def _get_prog():
    if "nc" not in _prog_cache:
        _prog_cache["nc"] = build_program(C=C, U=U, V=V, H2=H2, W=W, n_cores=8)
    return _prog_cache["nc"]


def make_in_maps(inputs):
    x = np.asarray(inputs["x"], dtype=np.float32)
    base = {
        "ident": np.eye(128, dtype=np.float32),
        "w1T": np.ascontiguousarray(np.asarray(inputs["w1"], np.float32).T),
        "w2T": np.ascontiguousarray(np.asarray(inputs["w2"], np.float32).T),
        "b1": np.asarray(inputs["b1"], np.float32).reshape(C, 1).copy(),
        "b2": np.asarray(inputs["b2"], np.float32).reshape(C, 1).copy(),
    }
    for i in range(4):
        base[f"f{i}T"] = np.ascontiguousarray(
            np.asarray(inputs[f"fw{i}"], np.float32).T)
        base[f"fb{i}"] = np.asarray(inputs[f"fb{i}"], np.float32).reshape(C, 1).copy()

    in_maps = []
    for core in range(8):
        b, hh = core // 2, core % 2
        shard = np.ascontiguousarray(x[b, :, :, :, hh * H2:(hh + 1) * H2, :])
        in_maps.append({"x": shard, **base})
    return in_maps


def assemble_out(results):
    out = np.empty((B, C, U, V, H, W), dtype=np.float32)
    for core in range(8):
        b, hh = core // 2, core % 2
        out[b, :, :, :, hh * H2:(hh + 1) * H2, :] = results[core]["out"]
    return out


def kernel(**inputs):
    from concourse.bass_utils import run_bass_kernel_spmd

    in_maps = make_in_maps(inputs)
    nc = _get_prog()
    res = run_bass_kernel_spmd(nc, in_maps, core_ids=list(range(8)))
    return assemble_out(res.results)
